# revision 19
# baseline (speedup 1.0000x reference)
"""HarmonicCausalSelfAttention on 8 TRN2 NeuronCores.

Sharding: core c -> (batch b = c//2, head-group g = c%2); each core computes
attention for 8 heads of one batch and a full-width partial of the output
projection; the host sums the two partials per batch.

v2 layout strategy (head-PAIR packed to fill the 128-wide PE array):
  stage1:  t^T[rank, T]   = A @ x^T          (x^T prepared host-side, bf16)
  stage2:  QTP/KTP[128, T] per pair: partitions 0-63 = head 2p, 64-127 = 2p+1
           (col-tiled matmul pairs, M=64 each, concurrent)
  attn:    S2[128, 1024] = both heads' scores for one 512-q chunk
           (row-tiled matmul pairs, K=64 each, concurrent)
           exp on ScalarE over [128, 512+cw] in one shot (gap cols unused)
           causal diag: post-exp multiply by 0/1 bf16 mask on GpSimd
           PV: pvt[65, h, j, 512] += [V_kb | 1]^T @ P^T  (row 64 = denom)
           normalize: reciprocal_approx_fast on [1, 2048] denoms,
           ones-matmul broadcast (col-tiled pair), DVE multiply
  c_proj:  r^T[rank, T] = sum_h cA_h @ Y^T_h ; out = r^T chunks.T @ cB^T
           (psum->sbuf copies for out on ScalarE, idle by then)
"""

import numpy as np
import ml_dtypes

import concourse.bass as bass
from concourse import bacc
import concourse.mybir as mybir
from concourse.tile import TileContext
from concourse.bass_utils import run_bass_kernel_spmd

B, T, C = 4, 2048, 1024
NH, HD = 16, 64
RANK = 128
NCORES = 8
HPC = 8          # heads per core
NPAIR = 4        # head pairs per core
G = 512          # C columns per head group
P = 128
F32 = mybir.dt.float32
BF16 = mybir.dt.bfloat16
BF = ml_dtypes.bfloat16

_NC_CACHE = None


def _chunks(total, step):
    res = []
    o = 0
    while o < total:
        res.append((o, min(step, total - o)))
        o += min(step, total - o)
    return res


def build():
    nc = bacc.Bacc()
    dp = nc.declare_dram_parameter
    xT = dp("xT", [C, T], BF16, isOutput=False)
    qAT = dp("qAT", [C, RANK], BF16, isOutput=False)
    kAT = dp("kAT", [C, RANK], BF16, isOutput=False)
    vAT = dp("vAT", [C, RANK], BF16, isOutput=False)
    qBT = dp("qBT", [RANK, G], BF16, isOutput=False)
    kBT = dp("kBT", [RANK, G], BF16, isOutput=False)
    vBT = dp("vBT", [RANK, G], BF16, isOutput=False)
    cAT = dp("cAT", [G, RANK], BF16, isOutput=False)
    cBT = dp("cBT", [RANK, C], BF16, isOutput=False)
    maskp = dp("mask", [P, P], BF16, isOutput=False)   # -30000 where k>q else 0
    identp = dp("ident", [P, P], BF16, isOutput=False)
    onesp = dp("ones64", [1, 64], F32, isOutput=False)
    out = dp("out", [T, C], F32, isOutput=True)

    Exp = mybir.ActivationFunctionType.Exp
    MULT = mybir.AluOpType.mult

    with TileContext(nc) as tc:
        with tc.tile_pool(name="sb", bufs=1) as sb:
            xT_sb = sb.tile([P, 8, T], BF16, tag="xT")
            nc.gpsimd.dma_start(out=xT_sb, in_=xT.rearrange("(co ci) t -> ci co t", ci=P))
            qAT_sb = sb.tile([P, 8, RANK], BF16, tag="qAT")
            nc.gpsimd.dma_start(out=qAT_sb, in_=qAT.rearrange("(co ci) r -> ci co r", ci=P))
            kAT_sb = sb.tile([P, 8, RANK], BF16, tag="kAT")
            nc.gpsimd.dma_start(out=kAT_sb, in_=kAT.rearrange("(co ci) r -> ci co r", ci=P))
            vAT_sb = sb.tile([P, 8, RANK], BF16, tag="vAT")
            nc.gpsimd.dma_start(out=vAT_sb, in_=vAT.rearrange("(co ci) r -> ci co r", ci=P))
            qBT_sb = sb.tile([RANK, G], BF16, tag="qBT")
            nc.gpsimd.dma_start(out=qBT_sb, in_=qBT[:, :])
            kBT_sb = sb.tile([RANK, G], BF16, tag="kBT")
            nc.gpsimd.dma_start(out=kBT_sb, in_=kBT[:, :])
            vBT_sb = sb.tile([RANK, G], BF16, tag="vBT")
            nc.gpsimd.dma_start(out=vBT_sb, in_=vBT[:, :])
            cAT_sb = sb.tile([64, HPC, RANK], BF16, tag="cAT")
            nc.gpsimd.dma_start(out=cAT_sb, in_=cAT.rearrange("(h d) r -> d h r", d=64))
            cBT_sb = sb.tile([RANK, C], BF16, tag="cBT")
            nc.gpsimd.dma_start(out=cBT_sb, in_=cBT[:, :])
            mask_sb = sb.tile([P, P], BF16, tag="mask")
            nc.gpsimd.dma_start(out=mask_sb, in_=maskp[:, :])
            ident_sb = sb.tile([P, P], BF16, tag="ident")
            nc.gpsimd.dma_start(out=ident_sb, in_=identp[:, :])
            ones_sb = sb.tile([1, 64], F32, tag="ones")
            nc.gpsimd.dma_start(out=ones_sb, in_=onesp[:, :])

            # pair-stacked Q^T/K^T: partitions 0-63 head 2p, 64-127 head 2p+1
            QTP = sb.tile([P, NPAIR, T], BF16, tag="QTP")
            KTP = sb.tile([P, NPAIR, T], BF16, tag="KTP")
            # Y^T per parity: Y0 = even heads, Y1 = odd heads
            Y0 = sb.tile([64, NPAIR, T], BF16, tag="Y0")
            Y1 = sb.tile([64, NPAIR, T], BF16, tag="Y1")
            V_sb = sb.tile([P, 16, HPC, 65], BF16, tag="Vsb")
            tTq = sb.tile([P, T], BF16, tag="tTq")
            tTk = sb.tile([P, T], BF16, tag="tTk")
            tTv = sb.tile([P, T], BF16, tag="tTv")
            rT_sb = sb.tile([P, T], BF16, tag="rT")

            nc.gpsimd.memset(V_sb[:, :, :, 64:65], 1.0)

            # ---- phase A: t^T = A @ x^T for q,k,v ----
            with (
                tc.tile_pool(name="psA", bufs=1, space="PSUM") as psA,
            ):
                for AT_sb, tT in ((qAT_sb, tTq), (kAT_sb, tTk), (vAT_sb, tTv)):
                    pt = psA.tile([P, T], F32, tag="psA")
                    for cc in range(8):
                        for t0, tw in _chunks(T, 512):
                            nc.tensor.matmul(
                                pt[:, t0:t0 + tw],
                                AT_sb[:, cc, :],
                                xT_sb[:, cc, t0:t0 + tw],
                                start=(cc == 0), stop=(cc == 7),
                            )
                    nc.scalar.copy(out=tT, in_=pt)

            # ---- attention (per head pair, row-tiled concurrent QK) ----
            with (
                tc.tile_pool(name="psS", bufs=2, space="PSUM") as psS,
                tc.tile_pool(name="psPV", bufs=1, space="PSUM") as psPV,
                tc.tile_pool(name="ptp", bufs=4) as ptp,
                tc.tile_pool(name="nrm", bufs=2) as nrm,
                tc.tile_pool(name="nr1", bufs=1) as nr1,
            ):
                for p in range(NPAIR):
                    # stage2 for this pair: pair-stacked Q^T/K^T (col-tiled)
                    for BT_sb, destP, tT in ((qBT_sb, QTP, tTq),
                                             (kBT_sb, KTP, tTk)):
                        for t0, tw in _chunks(T, 512):
                            p2 = psS.tile([P, 1024], F32, tag="s2")
                            for hh in range(2):
                                h = 2 * p + hh
                                nc.tensor.matmul(
                                    p2[hh * 64:(hh + 1) * 64, :tw],
                                    BT_sb[:, h * 64:(h + 1) * 64],
                                    tT[:, t0:t0 + tw],
                                    start=True, stop=True,
                                )
                            nc.vector.tensor_copy(
                                out=destP[:, p, t0:t0 + tw], in_=p2[:, :tw])
                    # V for this pair (keys-major, ones col pre-set)
                    for ti in range(16):
                        pvv = psS.tile([P, 1024], F32, tag="s2")
                        nc.tensor.matmul(
                            pvv[:, 0:128], tTv[:, ti * 128:(ti + 1) * 128],
                            vBT_sb[:, p * 128:(p + 1) * 128],
                            start=True, stop=True,
                        )
                        nc.vector.tensor_copy(
                            out=V_sb[:, ti, 2 * p:2 * p + 2, 0:64],
                            in_=pvv[:, 0:128].rearrange("p (h d) -> p h d", d=64),
                        )
                    for hc in range(2):
                        # pvt[65, hh, jj, 512]: accum over kb; row 64 = denom
                        pvt = psPV.tile([65, 2, 2, 512], F32, tag="pv",
                                        name=f"pv{p}_{hc}")
                        nkb = 8 if hc == 0 else 16
                        for kb in range(nkb):
                            qlo = max(1024 * hc, 128 * kb)
                            qhi = 1024 * (hc + 1)
                            w = qhi - qlo
                            for c0, cw in _chunks(w, 512):
                                s2 = psS.tile([P, 1024], F32, tag="s2")
                                diag = (c0 == 0 and qlo == 128 * kb)
                                for hh in range(2):
                                    nc.tensor.matmul(
                                        s2[:, hh * 512:hh * 512 + cw],
                                        KTP[hh * 64:(hh + 1) * 64, p,
                                            kb * 128:(kb + 1) * 128],
                                        QTP[hh * 64:(hh + 1) * 64, p,
                                            qlo + c0:qlo + c0 + cw],
                                        start=True, stop=True,
                                    )
                                ptile = ptp.tile([P, 1024], BF16, tag="pt")
                                width = 512 + cw
                                nc.scalar.activation(
                                    ptile[:, :width], s2[:, :width], Exp,
                                    scale=0.125)
                                if diag:
                                    for hh in range(2):
                                        o = hh * 512
                                        nc.vector.tensor_tensor(
                                            out=ptile[:, o:o + P],
                                            in0=ptile[:, o:o + P],
                                            in1=mask_sb, op=MULT)
                                # PV accumulate; split chunk at j boundary
                                a0 = qlo + c0 - 1024 * hc
                                pieces = []
                                if a0 < 512:
                                    pieces.append((0, a0, min(512 - a0, cw)))
                                if a0 + cw > 512:
                                    ps_ = max(a0, 512)
                                    pieces.append((1, ps_ - 512, a0 + cw - ps_))
                                for hh in range(2):
                                    for jj, r0, pw in pieces:
                                        j = 2 * hc + jj
                                        src = hh * 512 + (512 * jj + r0 - a0)
                                        nc.tensor.matmul(
                                            pvt[:, hh, jj, r0:r0 + pw],
                                            V_sb[:, kb, 2 * p + hh, :],
                                            ptile[:, src:src + pw],
                                            start=(kb == 0),
                                            stop=(kb == 4 * j + 3),
                                        )
                        # ---- normalize both heads, both 512-chunks ----
                        den0 = nr1.tile([1, 2048], F32, tag="den0")
                        nc.vector.tensor_copy(
                            out=den0,
                            in_=pvt[64:65, :, :, :].rearrange(
                                "o h j q -> o (h j q)"))
                        rec = nr1.tile([1, 2, 2, 512], F32, tag="rec")
                        nc.vector.reciprocal_approx_fast(
                            out=rec.rearrange("o h j q -> o (h j q)"),
                            in_=den0)
                        bcp = psS.tile([P, 1024], F32, tag="s2")
                        for jj in range(2):
                            for hh in range(2):
                                nc.tensor.matmul(
                                    bcp[hh * 64:(hh + 1) * 64,
                                        jj * 512:(jj + 1) * 512],
                                    ones_sb, rec[0:1, hh, jj, :],
                                    start=True, stop=True,
                                )
                        for hh in range(2):
                            bcs = nr1.tile([64, 1024], F32, tag=f"bcs{hh}")
                            nc.vector.tensor_copy(
                                out=bcs, in_=bcp[hh * 64:(hh + 1) * 64, :])
                            Yd = Y0 if hh == 0 else Y1
                            nc.vector.tensor_mul(
                                out=Yd[:, p, 1024 * hc:1024 * (hc + 1)],
                                in0=pvt[0:64, hh, :, :].rearrange(
                                    "p j q -> p (j q)"),
                                in1=bcs,
                            )

            # ---- phase D: c_proj ----
            with (
                tc.tile_pool(name="psD", bufs=1, space="PSUM") as psD,
                tc.tile_pool(name="psO", bufs=3, space="PSUM") as psO,
                tc.tile_pool(name="ost", bufs=3) as ost,
            ):
                pr = psD.tile([P, T], F32, tag="r")
                for h in range(HPC):
                    Yd = Y0 if h % 2 == 0 else Y1
                    for t0, tw in _chunks(T, 512):
                        nc.tensor.matmul(
                            pr[:, t0:t0 + tw], cAT_sb[:, h, :],
                            Yd[:, h // 2, t0:t0 + tw],
                            start=(h == 0), stop=(h == HPC - 1),
                        )
                nc.scalar.copy(out=rT_sb, in_=pr)
                for ti in range(16):
                    for nn in range(2):
                        po = psO.tile([P, 512], F32, tag="o")
                        nc.tensor.matmul(
                            po, rT_sb[:, ti * 128:(ti + 1) * 128],
                            cBT_sb[:, nn * 512:(nn + 1) * 512],
                            start=True, stop=True,
                        )
                        ob = ost.tile([P, 512], F32, tag="ob")
                        if (ti + nn) % 2 == 0:
                            nc.vector.tensor_copy(out=ob, in_=po)
                        else:
                            nc.scalar.copy(out=ob, in_=po)
                        nc.sync.dma_start(
                            out=out[ti * 128:(ti + 1) * 128, nn * 512:(nn + 1) * 512],
                            in_=ob,
                        )
    nc.finalize()
    return nc


def make_in_maps(x, qA, qB, kA, kB, vA, vB, cA, cB):
    x, qA, qB, kA, kB, vA, vB, cA, cB = [
        np.asarray(a, dtype=np.float32) for a in (x, qA, qB, kA, kB, vA, vB, cA, cB)
    ]
    mask30k = (np.arange(P)[:, None] <= np.arange(P)[None, :]).astype(BF)
    ident = np.eye(P, dtype=np.float32).astype(BF)
    ones64 = np.ones((1, 64), np.float32)
    qATn = np.ascontiguousarray(qA.T).astype(BF)
    kATn = np.ascontiguousarray(kA.T).astype(BF)
    vATn = np.ascontiguousarray(vA.T).astype(BF)
    cBTn = np.ascontiguousarray(cB.T).astype(BF)
    in_maps = []
    for c in range(NCORES):
        b, g = divmod(c, 2)
        sl = slice(g * G, (g + 1) * G)
        in_maps.append({
            "xT": np.ascontiguousarray(x[b].T).astype(BF),
            "qAT": qATn, "kAT": kATn, "vAT": vATn,
            "qBT": np.ascontiguousarray(qB[sl, :].T).astype(BF),
            "kBT": np.ascontiguousarray(kB[sl, :].T).astype(BF),
            "vBT": np.ascontiguousarray(vB[sl, :].T).astype(BF),
            "cAT": np.ascontiguousarray(cA[:, sl].T).astype(BF),
            "cBT": cBTn,
            "mask": mask30k, "ident": ident, "ones64": ones64,
        })
    return in_maps


def combine(parts):
    return np.stack(
        [parts[2 * b].astype(np.float32) + parts[2 * b + 1].astype(np.float32)
         for b in range(B)], axis=0)


def kernel(x, qA, qB, kA, kB, vA, vB, cA, cB):
    global _NC_CACHE
    if _NC_CACHE is None:
        _NC_CACHE = build()
    in_maps = make_in_maps(x, qA, qB, kA, kB, vA, vB, cA, cB)
    res = run_bass_kernel_spmd(_NC_CACHE, in_maps, list(range(NCORES))).results
    return combine([res[c]["out"] for c in range(NCORES)])


# revision 20
# speedup vs baseline: 1.0659x; 1.0659x over previous
"""HarmonicCausalSelfAttention on 8 TRN2 NeuronCores.

Sharding: core c -> (batch b = c//2, head-group g = c%2); each core computes
attention for 8 heads of one batch and a full-width partial of the output
projection; the host sums the two partials per batch.

v2 layout strategy (head-PAIR packed to fill the 128-wide PE array):
  stage1:  t^T[rank, T]   = A @ x^T          (x^T prepared host-side, bf16)
  stage2:  QTP/KTP[128, T] per pair: partitions 0-63 = head 2p, 64-127 = 2p+1
           (col-tiled matmul pairs, M=64 each, concurrent)
  attn:    S2[128, 1024] = both heads' scores for one 512-q chunk
           (row-tiled matmul pairs, K=64 each, concurrent)
           exp on ScalarE over [128, 512+cw] in one shot (gap cols unused)
           causal diag: post-exp multiply by 0/1 bf16 mask on GpSimd
           PV: pvt[65, h, j, 512] += [V_kb | 1]^T @ P^T  (row 64 = denom)
           normalize: reciprocal_approx_fast on [1, 2048] denoms,
           ones-matmul broadcast (col-tiled pair), DVE multiply
  c_proj:  r^T[rank, T] = sum_h cA_h @ Y^T_h ; out = r^T chunks.T @ cB^T
           (psum->sbuf copies for out on ScalarE, idle by then)
"""

import numpy as np
import ml_dtypes

import concourse.bass as bass
from concourse import bacc
import concourse.mybir as mybir
from concourse.tile import TileContext
from concourse.bass_utils import run_bass_kernel_spmd

B, T, C = 4, 2048, 1024
NH, HD = 16, 64
RANK = 128
NCORES = 8
HPC = 8          # heads per core
NPAIR = 4        # head pairs per core
G = 512          # C columns per head group
P = 128
F32 = mybir.dt.float32
BF16 = mybir.dt.bfloat16
BF = ml_dtypes.bfloat16

_NC_CACHE = None


def _chunks(total, step):
    res = []
    o = 0
    while o < total:
        res.append((o, min(step, total - o)))
        o += min(step, total - o)
    return res


def build():
    nc = bacc.Bacc()
    dp = nc.declare_dram_parameter
    xT = dp("xT", [C, T], BF16, isOutput=False)
    qAT = dp("qAT", [C, RANK], BF16, isOutput=False)
    kAT = dp("kAT", [C, RANK], BF16, isOutput=False)
    vAT = dp("vAT", [C, RANK], BF16, isOutput=False)
    qBT = dp("qBT", [RANK, G], BF16, isOutput=False)
    kBT = dp("kBT", [RANK, G], BF16, isOutput=False)
    vBT = dp("vBT", [RANK, G], BF16, isOutput=False)
    cAT = dp("cAT", [G, RANK], BF16, isOutput=False)
    cBT = dp("cBT", [RANK, C], BF16, isOutput=False)
    maskp = dp("mask", [P, P], BF16, isOutput=False)   # -30000 where k>q else 0
    identp = dp("ident", [P, P], BF16, isOutput=False)
    onesp = dp("ones64", [1, 64], F32, isOutput=False)
    out = dp("out", [T, C], F32, isOutput=True)

    Exp = mybir.ActivationFunctionType.Exp
    MULT = mybir.AluOpType.mult

    with TileContext(nc) as tc:
        with tc.tile_pool(name="sb", bufs=1) as sb:
            xT_sb = sb.tile([P, 8, T], BF16, tag="xT")
            nc.gpsimd.dma_start(out=xT_sb, in_=xT.rearrange("(co ci) t -> ci co t", ci=P))
            qAT_sb = sb.tile([P, 8, RANK], BF16, tag="qAT")
            nc.gpsimd.dma_start(out=qAT_sb, in_=qAT.rearrange("(co ci) r -> ci co r", ci=P))
            kAT_sb = sb.tile([P, 8, RANK], BF16, tag="kAT")
            nc.gpsimd.dma_start(out=kAT_sb, in_=kAT.rearrange("(co ci) r -> ci co r", ci=P))
            vAT_sb = sb.tile([P, 8, RANK], BF16, tag="vAT")
            nc.gpsimd.dma_start(out=vAT_sb, in_=vAT.rearrange("(co ci) r -> ci co r", ci=P))
            qBT_sb = sb.tile([RANK, G], BF16, tag="qBT")
            nc.gpsimd.dma_start(out=qBT_sb, in_=qBT[:, :])
            kBT_sb = sb.tile([RANK, G], BF16, tag="kBT")
            nc.gpsimd.dma_start(out=kBT_sb, in_=kBT[:, :])
            vBT_sb = sb.tile([RANK, G], BF16, tag="vBT")
            nc.gpsimd.dma_start(out=vBT_sb, in_=vBT[:, :])
            cAT_sb = sb.tile([64, HPC, RANK], BF16, tag="cAT")
            nc.gpsimd.dma_start(out=cAT_sb, in_=cAT.rearrange("(h d) r -> d h r", d=64))
            cBT_sb = sb.tile([RANK, C], BF16, tag="cBT")
            nc.gpsimd.dma_start(out=cBT_sb, in_=cBT[:, :])
            mask_sb = sb.tile([P, P], BF16, tag="mask")
            nc.gpsimd.dma_start(out=mask_sb, in_=maskp[:, :])
            ident_sb = sb.tile([P, P], BF16, tag="ident")
            nc.gpsimd.dma_start(out=ident_sb, in_=identp[:, :])
            ones_sb = sb.tile([1, 64], F32, tag="ones")
            nc.gpsimd.dma_start(out=ones_sb, in_=onesp[:, :])

            # pair-stacked Q^T/K^T: partitions 0-63 head 2p, 64-127 head 2p+1
            QTP = sb.tile([P, NPAIR, T], BF16, tag="QTP")
            KTP = sb.tile([P, NPAIR, T], BF16, tag="KTP")
            # Y^T per parity: Y0 = even heads, Y1 = odd heads
            Y0 = sb.tile([64, NPAIR, T], BF16, tag="Y0")
            Y1 = sb.tile([64, NPAIR, T], BF16, tag="Y1")
            V_sb = sb.tile([P, 16, HPC, 65], BF16, tag="Vsb")
            tTq = sb.tile([P, T], BF16, tag="tTq")
            tTk = sb.tile([P, T], BF16, tag="tTk")
            tTv = sb.tile([P, T], BF16, tag="tTv")
            rT_sb = sb.tile([P, T], BF16, tag="rT")

            nc.gpsimd.memset(V_sb[:, :, :, 64:65], 1.0)

            # ---- phase A: t^T = A @ x^T for q,k,v ----
            with (
                tc.tile_pool(name="psA", bufs=1, space="PSUM") as psA,
                tc.tile_pool(name="psB", bufs=2, space="PSUM") as psB,
                tc.tile_pool(name="psV", bufs=2, space="PSUM") as psV,
            ):
                for AT_sb, tT in ((qAT_sb, tTq), (kAT_sb, tTk), (vAT_sb, tTv)):
                    pt = psA.tile([P, T], F32, tag="psA")
                    for cc in range(8):
                        for t0, tw in _chunks(T, 512):
                            nc.tensor.matmul(
                                pt[:, t0:t0 + tw],
                                AT_sb[:, cc, :],
                                xT_sb[:, cc, t0:t0 + tw],
                                start=(cc == 0), stop=(cc == 7),
                            )
                    nc.scalar.copy(out=tT, in_=pt)

                # ---- phase B: pair-stacked Q^T/K^T via col-tiled matmul pairs
                for BT_sb, destP, tT in ((qBT_sb, QTP, tTq), (kBT_sb, KTP, tTk)):
                    for p in range(NPAIR):
                        for t0, tw in _chunks(T, 512):
                            p2 = psB.tile([P, 512], F32, tag="psB")
                            for hh in range(2):
                                h = 2 * p + hh
                                nc.tensor.matmul(
                                    p2[hh * 64:(hh + 1) * 64, :tw],
                                    BT_sb[:, h * 64:(h + 1) * 64],
                                    tT[:, t0:t0 + tw],
                                    start=True, stop=True,
                                )
                            nc.vector.tensor_copy(
                                out=destP[:, p, t0:t0 + tw], in_=p2[:, :tw])

                # ---- phase B: V keys-major with ones column ----
                for ti in range(16):
                    pv = psV.tile([P, G], F32, tag="psV")
                    nc.tensor.matmul(
                        pv, tTv[:, ti * 128:(ti + 1) * 128], vBT_sb,
                        start=True, stop=True,
                    )
                    nc.scalar.copy(
                        out=V_sb[:, ti, :, 0:64],
                        in_=pv.rearrange("p (h d) -> p h d", d=64),
                    )

            # ---- attention (per head pair, row-tiled concurrent QK) ----
            with (
                tc.tile_pool(name="psS", bufs=2, space="PSUM") as psS,
                tc.tile_pool(name="psPV", bufs=1, space="PSUM") as psPV,
                tc.tile_pool(name="ptp", bufs=4) as ptp,
                tc.tile_pool(name="nrm", bufs=2) as nrm,
                tc.tile_pool(name="nr1", bufs=1) as nr1,
            ):
                for p in range(NPAIR):
                    for hc in range(2):
                        # pvt[65, hh, jj, 512]: accum over kb; row 64 = denom
                        pvt = psPV.tile([65, 2, 2, 512], F32, tag="pv",
                                        name=f"pv{p}_{hc}")
                        nkb = 8 if hc == 0 else 16
                        for kb in range(nkb):
                            qlo = max(1024 * hc, 128 * kb)
                            qhi = 1024 * (hc + 1)
                            w = qhi - qlo
                            for c0, cw in _chunks(w, 512):
                                s2 = psS.tile([P, 1024], F32, tag="s2")
                                diag = (c0 == 0 and qlo == 128 * kb)
                                for hh in range(2):
                                    nc.tensor.matmul(
                                        s2[:, hh * 512:hh * 512 + cw],
                                        KTP[hh * 64:(hh + 1) * 64, p,
                                            kb * 128:(kb + 1) * 128],
                                        QTP[hh * 64:(hh + 1) * 64, p,
                                            qlo + c0:qlo + c0 + cw],
                                        start=True, stop=True,
                                    )
                                ptile = ptp.tile([P, 1024], BF16, tag="pt")
                                width = 512 + cw
                                nc.scalar.activation(
                                    ptile[:, :width], s2[:, :width], Exp,
                                    scale=0.125)
                                if diag:
                                    for hh in range(2):
                                        o = hh * 512
                                        nc.vector.tensor_tensor(
                                            out=ptile[:, o:o + P],
                                            in0=ptile[:, o:o + P],
                                            in1=mask_sb, op=MULT)
                                # PV accumulate; split chunk at j boundary
                                a0 = qlo + c0 - 1024 * hc
                                pieces = []
                                if a0 < 512:
                                    pieces.append((0, a0, min(512 - a0, cw)))
                                if a0 + cw > 512:
                                    ps_ = max(a0, 512)
                                    pieces.append((1, ps_ - 512, a0 + cw - ps_))
                                for hh in range(2):
                                    for jj, r0, pw in pieces:
                                        j = 2 * hc + jj
                                        src = hh * 512 + (512 * jj + r0 - a0)
                                        nc.tensor.matmul(
                                            pvt[:, hh, jj, r0:r0 + pw],
                                            V_sb[:, kb, 2 * p + hh, :],
                                            ptile[:, src:src + pw],
                                            start=(kb == 0),
                                            stop=(kb == 4 * j + 3),
                                        )
                        # ---- normalize both heads, both 512-chunks ----
                        den0 = nr1.tile([1, 2048], F32, tag="den0")
                        nc.vector.tensor_copy(
                            out=den0,
                            in_=pvt[64:65, :, :, :].rearrange(
                                "o h j q -> o (h j q)"))
                        rec = nr1.tile([1, 2, 2, 512], F32, tag="rec")
                        nc.vector.reciprocal_approx_fast(
                            out=rec.rearrange("o h j q -> o (h j q)"),
                            in_=den0)
                        bcp = psS.tile([P, 1024], F32, tag="s2")
                        for jj in range(2):
                            for hh in range(2):
                                nc.tensor.matmul(
                                    bcp[hh * 64:(hh + 1) * 64,
                                        jj * 512:(jj + 1) * 512],
                                    ones_sb, rec[0:1, hh, jj, :],
                                    start=True, stop=True,
                                )
                        for hh in range(2):
                            bcs = nr1.tile([64, 1024], F32, tag=f"bcs{hh}")
                            nc.vector.tensor_copy(
                                out=bcs, in_=bcp[hh * 64:(hh + 1) * 64, :])
                            Yd = Y0 if hh == 0 else Y1
                            nc.vector.tensor_mul(
                                out=Yd[:, p, 1024 * hc:1024 * (hc + 1)],
                                in0=pvt[0:64, hh, :, :].rearrange(
                                    "p j q -> p (j q)"),
                                in1=bcs,
                            )

            # ---- phase D: c_proj ----
            with (
                tc.tile_pool(name="psD", bufs=1, space="PSUM") as psD,
                tc.tile_pool(name="psO", bufs=3, space="PSUM") as psO,
                tc.tile_pool(name="ost", bufs=3) as ost,
            ):
                pr = psD.tile([P, T], F32, tag="r")
                for h in range(HPC):
                    Yd = Y0 if h % 2 == 0 else Y1
                    for t0, tw in _chunks(T, 512):
                        nc.tensor.matmul(
                            pr[:, t0:t0 + tw], cAT_sb[:, h, :],
                            Yd[:, h // 2, t0:t0 + tw],
                            start=(h == 0), stop=(h == HPC - 1),
                        )
                nc.scalar.copy(out=rT_sb, in_=pr)
                for ti in range(16):
                    for nn in range(2):
                        po = psO.tile([P, 512], F32, tag="o")
                        nc.tensor.matmul(
                            po, rT_sb[:, ti * 128:(ti + 1) * 128],
                            cBT_sb[:, nn * 512:(nn + 1) * 512],
                            start=True, stop=True,
                        )
                        ob = ost.tile([P, 512], F32, tag="ob")
                        if (ti + nn) % 2 == 0:
                            nc.vector.tensor_copy(out=ob, in_=po)
                        else:
                            nc.scalar.copy(out=ob, in_=po)
                        nc.sync.dma_start(
                            out=out[ti * 128:(ti + 1) * 128, nn * 512:(nn + 1) * 512],
                            in_=ob,
                        )
    nc.finalize()
    return nc


def make_in_maps(x, qA, qB, kA, kB, vA, vB, cA, cB):
    x, qA, qB, kA, kB, vA, vB, cA, cB = [
        np.asarray(a, dtype=np.float32) for a in (x, qA, qB, kA, kB, vA, vB, cA, cB)
    ]
    mask30k = (np.arange(P)[:, None] <= np.arange(P)[None, :]).astype(BF)
    ident = np.eye(P, dtype=np.float32).astype(BF)
    ones64 = np.ones((1, 64), np.float32)
    qATn = np.ascontiguousarray(qA.T).astype(BF)
    kATn = np.ascontiguousarray(kA.T).astype(BF)
    vATn = np.ascontiguousarray(vA.T).astype(BF)
    cBTn = np.ascontiguousarray(cB.T).astype(BF)
    in_maps = []
    for c in range(NCORES):
        b, g = divmod(c, 2)
        sl = slice(g * G, (g + 1) * G)
        in_maps.append({
            "xT": np.ascontiguousarray(x[b].T).astype(BF),
            "qAT": qATn, "kAT": kATn, "vAT": vATn,
            "qBT": np.ascontiguousarray(qB[sl, :].T).astype(BF),
            "kBT": np.ascontiguousarray(kB[sl, :].T).astype(BF),
            "vBT": np.ascontiguousarray(vB[sl, :].T).astype(BF),
            "cAT": np.ascontiguousarray(cA[:, sl].T).astype(BF),
            "cBT": cBTn,
            "mask": mask30k, "ident": ident, "ones64": ones64,
        })
    return in_maps


def combine(parts):
    return np.stack(
        [parts[2 * b].astype(np.float32) + parts[2 * b + 1].astype(np.float32)
         for b in range(B)], axis=0)


def kernel(x, qA, qB, kA, kB, vA, vB, cA, cB):
    global _NC_CACHE
    if _NC_CACHE is None:
        _NC_CACHE = build()
    in_maps = make_in_maps(x, qA, qB, kA, kB, vA, vB, cA, cB)
    res = run_bass_kernel_spmd(_NC_CACHE, in_maps, list(range(NCORES))).results
    return combine([res[c]["out"] for c in range(NCORES)])
